# revision 1
# baseline (speedup 1.0000x reference)
"""BinaryTreeRNN Trainium2 kernel — 8-core data-parallel.

Contract: kernel(**inputs) takes FULL unsharded inputs (x [4M,16] f32 plus tiny
tree params) and returns the FULL [4M] f32 output.

Design (per core, N_core = 500k samples, padded to 501760 = 560 blocks x 896):
  * Host folds all tree parameters:  softmax(om) -> per-node (A, P, R, phi, B);
    S*sin(s)+C*cos(s) == R*sin(s+phi).  Level-(l-1) phases are pushed into the
    children's outputs (out' = out + phi_parent/2) with exact algebraic
    compensation in the combine coefficients.
  * Host packs x into per-block stationary tiles xt[blk] = [128, 128]:
    rows 16a+v (a<7) hold x[blk*896 + 7p + a, v] in column p; rows 112..127
    hold 1.0 (constant slot -> every matmul column gets a free additive bias).
  * PE: per block, one fp32 matmul  out[p, c] = sum_k xt[k, p] * Wpat[k, c]
    producing 12 functions x 7 sample-slots = 84 columns, sample-major:
    cols [0,28) = biased left leaves (nodes 0..3), [28,56) biased right
    leaves, [56,84) = s3C = l+r+phi3 (sin-ready sums).
  * DVE/ACT tree: products via tensor_mul; range reduction via the
    round-to-nearest MAGIC trick  sc = s/2pi; k' = sc+MAGIC; f = (k'-MAGIC)-sc;
    sin(s+phi) = Sin(f * -2pi) on the scalar engine; combine via
    ln_bwd_dx (R*t + A*s + beta in one op) + affine_then_add (+P*p).
"""

import os
import sys

for _p in ("/opt/trn_rl_repo", "/root/.axon_site/_ro/trn_rl_repo"):
    if os.path.isdir(_p) and _p not in sys.path:
        sys.path.append(_p)

import numpy as np

N_FULL = 4_000_000
V = 16
N_CORES = 8
N_CORE = N_FULL // N_CORES          # 500_000
SLOTS = 7                            # samples per stationary column
BLK = 128 * SLOTS                    # 896 samples per matmul block
N_BLOCKS = 560                       # ceil(500000/896) -> padded
N_PAD = N_BLOCKS * BLK               # 501_760
B = 16                               # blocks per super-tile
N_ST = N_BLOCKS // B                 # 35

MAGIC = float(np.float32(1.5 * 2**23))
INV2PI = float(np.float32(1.0 / (2.0 * np.pi)))
NEG2PI = float(np.float32(-2.0 * np.pi))

F32 = np.float32


def _softmax64(om):
    e = np.exp(om.astype(np.float64) - om.astype(np.float64).max(-1, keepdims=True))
    return e / e.sum(-1, keepdims=True)


def _fold(leaf_w, leaf_b, w1, b1, om1, w2, b2, om2, w3, b3, om3):
    """float64 constant folding. Returns per-level dicts + matmul pattern."""
    lv = {}
    for lvl, (w, b, om) in {3: (w3, b3, om3), 2: (w2, b2, om2), 1: (w1, b1, om1)}.items():
        sm = _softmax64(om)
        w64 = w.astype(np.float64)
        A = w64 * sm[:, 0]
        S = w64 * sm[:, 1]
        C = w64 * sm[:, 2]
        P = w64 * sm[:, 3]
        R = np.hypot(S, C)
        phi = np.arctan2(C, S)
        lv[lvl] = dict(A=A, B=b.astype(np.float64), P=P, R=R, phi=phi)

    # combine: out = Aeff*sC + R*t + P*p + beta, via
    #   u0 = p*P + beta (TS); u1 = t*R + u0 (STT); out = sC*Aeff + u1 (STT)
    def cparams(Aeff, beta, R, P):
        return dict(A=float(F32(Aeff)), beta=float(F32(beta)),
                    R=float(F32(R)), P=float(F32(P)))

    c3, c2, c1 = lv[3], lv[2], lv[1]
    L3 = []
    for n in range(4):
        delta = c2["phi"][n // 2] / 2.0
        beta = c3["B"][n] - c3["A"][n] * c3["phi"][n] + delta
        L3.append(cparams(c3["A"][n], beta, c3["R"][n], c3["P"][n]))
    L2 = []
    for m in range(2):
        ph = c2["phi"][m]
        Aeff = c2["A"][m] - c2["P"][m] * ph / 2.0
        delta = c1["phi"][0] / 2.0
        beta = c2["B"][m] - c2["A"][m] * ph + c2["P"][m] * ph * ph / 4.0 + delta
        L2.append(cparams(Aeff, beta, c2["R"][m], c2["P"][m]))
    ph = c1["phi"][0]
    Aeff = c1["A"][0] - c1["P"][0] * ph / 2.0
    beta = c1["B"][0] - c1["A"][0] * ph + c1["P"][0] * ph * ph / 4.0
    L1 = [cparams(Aeff, beta, c1["R"][0], c1["P"][0])]

    # Wpat [128, 84]: col 7j+a, j = 0..11 functions, a = 0..6 slots.
    # rows 16a+v: weight of x[., v] for slot a;  rows 112+v: constant (only v=0 used).
    wp = np.zeros((128, 84), np.float64)
    lw = leaf_w.astype(np.float64)
    lb = leaf_b.astype(np.float64)
    for n in range(4):
        funcs = [
            (n, lw[2 * n], lb[2 * n]),                               # hl'
            (4 + n, lw[2 * n + 1], lb[2 * n + 1]),                   # hr'
            (8 + n, lw[2 * n] + lw[2 * n + 1],
             lb[2 * n] + lb[2 * n + 1] + c3["phi"][n]),              # s3C
        ]
        for j, wv, bias in funcs:
            for a in range(SLOTS):
                wp[16 * a: 16 * a + 16, 7 * j + a] = wv
                wp[112, 7 * j + a] = bias
    wp32 = wp  # float64
    wph = wp32.astype(np.float16)
    wpl = (wp32 - wph.astype(np.float64)).astype(np.float16)
    wp2 = np.concatenate([wph, wpl], axis=1)       # [128, 168] fp16
    return L3, L2, L1, wp2


def _pack_x(x_shard, n_st=N_ST, b_blocks=B):
    """[n, 16] f32 -> (xt_hi, xt_lo) fp16 [n_st, 128, b_blocks*128]:
    split-fp16 stationary tiles, per-partition supertile-contiguous lines."""
    npad = n_st * b_blocks * BLK
    xs = np.empty((npad, V), F32)
    xs[:len(x_shard)] = x_shard
    xs[len(x_shard):] = 1.0
    a = xs.reshape(n_st, b_blocks, 128, SLOTS, V)  # [st, b, p, a, v]
    xt = np.empty((n_st, 128, b_blocks, 128), F32)
    xt[:, :112] = a.transpose(0, 3, 4, 1, 2).reshape(n_st, 112, b_blocks, 128)
    xt[:, 112:] = 1.0
    xt = xt.reshape(n_st, 128, b_blocks * 128)
    xh = xt.astype(np.float16)
    xl = (xt - xh.astype(F32)).astype(np.float16)
    return xh, xl


_PROGRAM_CACHE = {}


def _build_program(n_st=N_ST, b_blocks=B):
    """Build + compile the per-core Bass program (identical on all cores)."""
    import json
    key = (n_st, b_blocks, json.dumps(_build_program.consts, sort_keys=True, default=str))
    if key in _PROGRAM_CACHE:
        return _PROGRAM_CACHE[key]

    import concourse.bass as bass
    import concourse.tile as tile
    from concourse import bacc, mybir
    from contextlib import ExitStack

    f32 = mybir.dt.float32
    Sin = mybir.ActivationFunctionType.Sin
    sub = mybir.AluOpType.subtract
    mult = mybir.AluOpType.mult
    addop = mybir.AluOpType.add
    nb = n_st * b_blocks

    nc = bacc.Bacc("TRN2", target_bir_lowering=False, debug=False,
                   num_devices=N_CORES)
    f16 = mybir.dt.float16
    xh_d = nc.dram_tensor("xh", [n_st, 128, b_blocks * 128], f16,
                          kind="ExternalInput")
    xl_d = nc.dram_tensor("xl", [n_st, 128, b_blocks * 128], f16,
                          kind="ExternalInput")
    wp_d = nc.dram_tensor("wp", [128, 168], f16, kind="ExternalInput")
    out_d = nc.dram_tensor("out", [n_st, 128, b_blocks, SLOTS], f32,
                           kind="ExternalOutput")

    # Constants are baked as immediates; read them from the module-level holder.
    L3, L2, L1 = _build_program.consts
    Ident = mybir.ActivationFunctionType.Identity
    GROUP = 5

    # Activation float biases require pre-registered const APs.
    def reg_const(v):
        if (f32, v) not in nc.const_aps.aps:
            t = nc.alloc_sbuf_tensor(
                f"constx-{len(nc.const_aps.aps)}", [128, 1], f32)
            nc.gpsimd.memset(t.ap(), v)
            nc.const_aps.aps[(f32, v)] = t.ap()

    for cn in L3 + L2 + L1:
        reg_const(cn["beta"])
    nc.all_engine_barrier()

    with tile.TileContext(nc) as tc:
        with ExitStack() as ctx:
            const_pool = ctx.enter_context(tc.tile_pool(name="const", bufs=1))
            xpool = ctx.enter_context(tc.tile_pool(name="x", bufs=2))
            ppool = ctx.enter_context(
                tc.tile_pool(name="ps", bufs=2, space=bass.MemorySpace.PSUM))
            wpool = ctx.enter_context(tc.tile_pool(name="w", bufs=2))
            gpool = ctx.enter_context(tc.tile_pool(name="g", bufs=2))

            wp = const_pool.tile([128, 168], f16)
            nc.sync.dma_start(wp[:], wp_d[:])

            def tt(pool, cols, nm):
                t = pool.tile([128, cols], f32, name=nm, tag=nm)
                return t, t[:].rearrange("p (b c) -> p b c", c=ccols[nm])

            st0 = 0
            while st0 < n_st:
                glen = min(GROUP, n_st - st0)
                q = glen * b_blocks
                ccols = {"hrc": 28, "s3Cg": 28, "p3g": 28, "sc3g": 28,
                         "k3g": 28, "f3g": 28, "t3g": 28, "u0g": 28,
                         "u1g": 28, "o3acc": 28,
                         "s2": 14, "p2": 14, "sc2": 14, "k2": 14,
                         "f2": 14, "t2": 14, "u0_2": 14, "u1_2": 14, "o2": 14,
                         "s1": 7, "p1": 7, "sc1": 7, "k1": 7, "f1": 7,
                         "t1": 7, "u0_1": 7, "u1_1": 7, "yo": 7}

                def gt(nm, bufs=1):
                    c = ccols[nm]
                    t = gpool.tile([128, GROUP * b_blocks * c], f32,
                                   name=nm, tag=nm, bufs=bufs)
                    return t, t[:].rearrange("p (q c) -> p q c", c=c)

                s3Cg, s3Cgv = gt("s3Cg", bufs=2)
                p3g, p3gv = gt("p3g", bufs=2)
                o3acc, o3accv = gt("o3acc")

                for seg in range(glen):
                    st = st0 + seg
                    x2h = xpool.tile([128, b_blocks * 128], f16, name="x2h",
                                     tag="x2h")
                    nc.sync.dma_start(x2h[:], xh_d[st])
                    x2l = xpool.tile([128, b_blocks * 128], f16, name="x2l",
                                     tag="x2l")
                    nc.sync.dma_start(x2l[:], xl_d[st])

                    ps = ppool.tile([128, b_blocks * 128], f32)
                    for b in range(b_blocks):
                        o = ps[:, 128 * b:128 * b + 84]
                        xhb = x2h[:, 128 * b:128 * b + 128]
                        xlb = x2l[:, 128 * b:128 * b + 128]
                        nc.tensor.matmul(o, xhb, wp[:, 0:84],
                                         start=True, stop=False)
                        nc.tensor.matmul(o, xhb, wp[:, 84:168],
                                         start=False, stop=False)
                        nc.tensor.matmul(o, xlb, wp[:, 0:84],
                                         start=False, stop=True)
                    psv = ps[:].rearrange("p (b c) -> p b c", c=128)
                    segsl = slice(seg * b_blocks, (seg + 1) * b_blocks)

                    # stage hr (per-st) + s3C (group buffer) in SBUF via ACT
                    hrc = wpool.tile([128, b_blocks * 28], f32, name="hrc",
                                     tag="hrc")
                    hrcv = hrc[:].rearrange("p (b c) -> p b c", c=28)
                    nc.scalar.copy(hrcv, psv[:, :, 28:56])
                    nc.scalar.copy(s3Cgv[:, segsl, :], psv[:, :, 56:84])
                    nc.vector.tensor_mul(p3gv[:, segsl, :],
                                         psv[:, :, 0:28], hrcv)

                # ---- level 3 (batched over the group) ----
                qf28 = q * 28
                sc3g, _ = gt("sc3g")
                nc.vector.tensor_scalar_mul(sc3g[:, 0:qf28], s3Cg[:, 0:qf28],
                                            INV2PI)
                k3g, _ = gt("k3g")
                nc.vector.tensor_scalar_add(k3g[:, 0:qf28], sc3g[:, 0:qf28],
                                            MAGIC)
                f3g, _ = gt("f3g")
                nc.vector.scalar_tensor_tensor(f3g[:, 0:qf28], k3g[:, 0:qf28],
                                               MAGIC, sc3g[:, 0:qf28],
                                               sub, sub)
                t3g, t3gv = gt("t3g")
                nc.scalar.activation(t3g[:, 0:qf28], f3g[:, 0:qf28], Sin,
                                     bias=0.0, scale=NEG2PI)
                u0g, u0gv = gt("u0g")
                u1g, u1gv = gt("u1g")
                # L2 pairing: l2-run = [o3_0, o3_2], r2-run = [o3_1, o3_3]
                opos = {0: 0, 2: 7, 1: 14, 3: 21}
                for n in range(4):
                    cn = L3[n]
                    sl = (slice(None), slice(0, q), slice(7 * n, 7 * n + 7))
                    nc.scalar.activation(u0gv[sl], p3gv[sl], Ident,
                                         bias=cn["beta"], scale=cn["P"])
                    nc.vector.scalar_tensor_tensor(
                        u1gv[sl], t3gv[sl], cn["R"], u0gv[sl], mult, addop)
                    nc.vector.scalar_tensor_tensor(
                        o3accv[:, 0:q, opos[n]:opos[n] + 7], s3Cgv[sl],
                        cn["A"], u1gv[sl], mult, addop)

                # ---- level 2 + level 1 (batched over the group) ----
                l2 = o3accv[:, 0:q, 0:14]
                r2 = o3accv[:, 0:q, 14:28]
                s2, s2f = gt("s2")
                s2v = s2f[:, 0:q, :]
                nc.gpsimd.tensor_add(s2v, l2, r2)
                p2, p2f = gt("p2")
                p2v = p2f[:, 0:q, :]
                nc.gpsimd.tensor_mul(p2v, l2, r2)
                qf = q * 14
                sc2, _ = gt("sc2")
                nc.vector.tensor_scalar_mul(sc2[:, 0:qf], s2[:, 0:qf], INV2PI)
                k2, _ = gt("k2")
                nc.vector.tensor_scalar_add(k2[:, 0:qf], sc2[:, 0:qf], MAGIC)
                f2, _ = gt("f2")
                nc.vector.scalar_tensor_tensor(f2[:, 0:qf], k2[:, 0:qf], MAGIC,
                                               sc2[:, 0:qf], sub, sub)
                t2, t2f = gt("t2")
                t2v = t2f[:, 0:q, :]
                nc.scalar.activation(t2[:, 0:qf], f2[:, 0:qf], Sin, bias=0.0,
                                     scale=NEG2PI)
                u0_2, u0_2f = gt("u0_2")
                u0_2v = u0_2f[:, 0:q, :]
                u1_2, u1_2f = gt("u1_2")
                u1_2v = u1_2f[:, 0:q, :]
                o2, o2f = gt("o2")
                o2v = o2f[:, 0:q, :]
                for m in range(2):
                    cm = L2[m]
                    sl = (slice(None), slice(0, q), slice(7 * m, 7 * m + 7))
                    nc.scalar.activation(u0_2f[sl], p2f[sl], Ident,
                                         bias=cm["beta"], scale=cm["P"])
                    nc.vector.scalar_tensor_tensor(
                        u1_2f[sl], t2f[sl], cm["R"], u0_2f[sl], mult, addop)
                    nc.vector.scalar_tensor_tensor(
                        o2f[sl], s2f[sl], cm["A"], u1_2f[sl], mult, addop)
                l1 = o2v[:, :, 0:7]
                r1 = o2v[:, :, 7:14]
                qf = q * 7
                s1, s1f = gt("s1")
                s1v = s1f[:, 0:q, :]
                nc.gpsimd.tensor_add(s1v, l1, r1)
                p1, p1f = gt("p1")
                p1v = p1f[:, 0:q, :]
                nc.gpsimd.tensor_mul(p1v, l1, r1)
                sc1, _ = gt("sc1")
                nc.vector.tensor_scalar_mul(sc1[:, 0:qf], s1[:, 0:qf], INV2PI)
                k1, _ = gt("k1")
                nc.vector.tensor_scalar_add(k1[:, 0:qf], sc1[:, 0:qf], MAGIC)
                f1, _ = gt("f1")
                nc.vector.scalar_tensor_tensor(f1[:, 0:qf], k1[:, 0:qf], MAGIC,
                                               sc1[:, 0:qf], sub, sub)
                t1, t1f = gt("t1")
                t1v = t1f[:, 0:q, :]
                nc.scalar.activation(t1[:, 0:qf], f1[:, 0:qf], Sin, bias=0.0,
                                     scale=NEG2PI)
                c1 = L1[0]
                u0_1, u0_1f = gt("u0_1")
                u0_1v = u0_1f[:, 0:q, :]
                nc.scalar.activation(u0_1v, p1v, Ident, bias=c1["beta"],
                                     scale=c1["P"])
                u1_1, u1_1f = gt("u1_1")
                u1_1v = u1_1f[:, 0:q, :]
                nc.vector.scalar_tensor_tensor(
                    u1_1v, t1v, c1["R"], u0_1v, mult, addop)
                yo, yof = gt("yo")
                yov = yof[:, 0:q, :]
                nc.vector.scalar_tensor_tensor(
                    yov, s1v, c1["A"], u1_1v, mult, addop)

                dst = out_d[st0:st0 + glen].transpose([1, 0, 2, 3])
                yo4 = yo[:, 0:qf].rearrange("p (g b a) -> p g b a",
                                            g=glen, a=SLOTS)
                nc.sync.dma_start(dst, yo4)
                st0 += glen

    nc.compile()
    _PROGRAM_CACHE[key] = nc
    return nc


def kernel(x, leaf_w, leaf_b, w1, b1, om1, w2, b2, om2, w3, b3, om3):
    from concourse.bass_interp import get_hw_module
    from concourse.bass_utils import run_bass_kernel_spmd

    L3, L2, L1, wp = _fold(leaf_w, leaf_b, w1, b1, om1, w2, b2, om2, w3, b3, om3)
    _build_program.consts = (L3, L2, L1)
    nc = _build_program()

    in_maps = []
    x = np.ascontiguousarray(x, dtype=F32)
    for c in range(N_CORES):
        xh, xl = _pack_x(x[c * N_CORE:(c + 1) * N_CORE])
        in_maps.append({"xh": xh, "xl": xl, "wp": wp})

    kw = {}
    if os.environ.get("KERNEL_TRACE_DIR"):
        kw["tmpdir"] = os.environ["KERNEL_TRACE_DIR"]
    old = nc.m
    nc.m = get_hw_module(nc.m)
    try:
        res = run_bass_kernel_spmd(nc, in_maps, core_ids=list(range(N_CORES)), **kw)
    finally:
        nc.m = old
    kernel._last = res

    out = np.empty(N_FULL, F32)
    for c in range(N_CORES):
        oc = res.results[c]["out"]          # [N_ST, 128, B, 7]
        oc = oc.transpose(0, 2, 1, 3).reshape(-1)[:N_CORE]
        out[c * N_CORE:(c + 1) * N_CORE] = oc
    return out



# revision 5
# speedup vs baseline: 1.4771x; 1.4771x over previous
"""BinaryTreeRNN Trainium2 kernel — 8-core data-parallel, v3.

Contract: kernel(**inputs) takes FULL unsharded inputs (x [4M,16] f32 plus tiny
tree params) and returns the FULL [4M] f32 output.

Design (per core, N_core = 500k samples, padded to 501760 = 560 blocks x 896):
  * x is packed host-side to fp16 only (empirically costs ~1.2e-3 rel err vs
    the 2e-2 gate; the output L2 norm is dominated by the product chain, so
    fp16 leaf precision is plenty).  One matmul stream per block.
  * Matmul emits 12 function columns x 7 slots = 84 cols per 128-sample block:
      col1_n = m_n*(hl+hr) + c_n      (m = sqrt(|P|)/2, c = A/(2m), signed)
      col2_n = m_n*(hl-hr)
      sc3_n  = INV2PI*(hl+hr+phi3)
    ACT Square(col1) = P/4*s3^2 + A*s3 + c^2 and Square(col2) = P/4*d3^2 give
    P*p3 + A*s3 via one subtract, since p3 = (s3^2-d3^2)/4.  The whole L3
    combine is then z = R*t3 + qa (STT), o3 = z + K - qb (STT) with
    K = beta -+ c^2 folded host-side.
  * Range reduction per level: k = fl(s*INV2PI + MAGIC) (one 2-scalar DVE
    tensor_scalar, RNE via the magic constant; at L3 the ACT does it from the
    PSUM sc3 column), f = (k-MAGIC) - sc (STT), t = Sin(-2pi*f) on ACT.
  * L2/L1: s/p on gpsimd, u0 = P*p + beta as 2-scalar DVE tensor_scalar,
    u1/o as STT chains.  Identical folding/push-down algebra to the reference.
"""

import os
import sys

for _p in ("/opt/trn_rl_repo", "/root/.axon_site/_ro/trn_rl_repo"):
    if os.path.isdir(_p) and _p not in sys.path:
        sys.path.append(_p)

import numpy as np

N_FULL = 4_000_000
V = 16
N_CORES = 8
N_CORE = N_FULL // N_CORES          # 500_000
SLOTS = 7                            # samples per stationary column
BLK = 128 * SLOTS                    # 896 samples per matmul block
N_BLOCKS = 560                       # ceil(500000/896) -> padded
N_PAD = N_BLOCKS * BLK               # 501_760
B = 16                               # blocks per super-tile
N_ST = N_BLOCKS // B                 # 35

MAGIC = float(np.float32(1.5 * 2**23))
INV2PI = float(np.float32(1.0 / (2.0 * np.pi)))
NEG2PI = float(np.float32(-2.0 * np.pi))

F32 = np.float32


def _softmax64(om):
    e = np.exp(om.astype(np.float64) - om.astype(np.float64).max(-1, keepdims=True))
    return e / e.sum(-1, keepdims=True)


def _fold(leaf_w, leaf_b, w1, b1, om1, w2, b2, om2, w3, b3, om3):
    """float64 constant folding. Returns per-level consts + matmul pattern."""
    lv = {}
    for lvl, (w, b, om) in {3: (w3, b3, om3), 2: (w2, b2, om2), 1: (w1, b1, om1)}.items():
        sm = _softmax64(om)
        w64 = w.astype(np.float64)
        A = w64 * sm[:, 0]
        S = w64 * sm[:, 1]
        C = w64 * sm[:, 2]
        P = w64 * sm[:, 3]
        R = np.hypot(S, C)
        phi = np.arctan2(C, S)
        lv[lvl] = dict(A=A, B=b.astype(np.float64), P=P, R=R, phi=phi)

    def cparams(Aeff, beta, R, P):
        return dict(A=float(F32(Aeff)), beta=float(F32(beta)),
                    R=float(F32(R)), P=float(F32(P)))

    c3, c2, c1 = lv[3], lv[2], lv[1]
    # L3: o3'_n = A*s3 + P*p3 + R*sin(s3+phi3) + beta3,
    #     beta3 = B - A*phi3 + delta ... NOTE the baseline folded A*s3C - A*phi3;
    #     here A multiplies RAW s3 directly so beta3 = B + delta.
    L3 = []
    wp = np.zeros((128, 84), np.float64)
    lw = leaf_w.astype(np.float64)
    lb = leaf_b.astype(np.float64)
    for n in range(4):
        delta = c2["phi"][n // 2] / 2.0
        A, P, R, beta = c3["A"][n], c3["P"][n], c3["R"][n], c3["B"][n] + delta
        absP = abs(P)
        assert absP > 1e-12, f"P3[{n}] ~ 0, square-trick guard tripped"
        m = np.sqrt(absP) / 2.0
        c = A / (2.0 * m)
        assert abs(c) < 5e3, f"|c3[{n}]| = {abs(c):.3g} too large (cancellation)"
        sgn = 1.0 if P >= 0 else -1.0
        # col1 = m*s3 + sgn*c ;  q1 = col1^2
        # P>=0: o3 =  q1 - q2 + R*t3 + (beta - c^2)   (qa=q1, qb=q2)
        # P<0 : o3 = -q1 + q2 + R*t3 + (beta + c^2)   (qa=q2, qb=q1)
        K = beta - sgn * c * c
        L3.append(dict(qa=(0 if sgn > 0 else 1), K=float(F32(K)), R=float(F32(R))))
        ws = lw[2 * n] + lw[2 * n + 1]
        wd = lw[2 * n] - lw[2 * n + 1]
        bs = lb[2 * n] + lb[2 * n + 1]
        bd = lb[2 * n] - lb[2 * n + 1]
        for a in range(SLOTS):
            wp[16 * a: 16 * a + 16, 7 * (0 + n) + a] = m * ws
            wp[112, 7 * (0 + n) + a] = m * bs + sgn * c
            wp[16 * a: 16 * a + 16, 7 * (4 + n) + a] = m * wd
            wp[112, 7 * (4 + n) + a] = m * bd
            wp[16 * a: 16 * a + 16, 7 * (8 + n) + a] = INV2PI * ws
            wp[112, 7 * (8 + n) + a] = INV2PI * (bs + c3["phi"][n])

    L2 = []
    for mm in range(2):
        ph = c2["phi"][mm]
        Aeff = c2["A"][mm] - c2["P"][mm] * ph / 2.0
        delta = c1["phi"][0] / 2.0
        beta = c2["B"][mm] - c2["A"][mm] * ph + c2["P"][mm] * ph * ph / 4.0 + delta
        L2.append(cparams(Aeff, beta, c2["R"][mm], c2["P"][mm]))
    ph = c1["phi"][0]
    Aeff = c1["A"][0] - c1["P"][0] * ph / 2.0
    beta = c1["B"][0] - c1["A"][0] * ph + c1["P"][0] * ph * ph / 4.0
    L1 = [cparams(Aeff, beta, c1["R"][0], c1["P"][0])]

    return L3, L2, L1, wp.astype(np.float16)


def _pack_x(x_shard, n_st=N_ST, b_blocks=B):
    """[n, 16] f32 -> fp16 [n_st, 128, b_blocks*128] stationary tiles."""
    npad = n_st * b_blocks * BLK
    xs = np.empty((npad, V), F32)
    xs[:len(x_shard)] = x_shard
    xs[len(x_shard):] = 1.0
    a = xs.reshape(n_st, b_blocks, 128, SLOTS, V)  # [st, b, p, a, v]
    xt = np.empty((n_st, 128, b_blocks, 128), np.float16)
    xt[:, :112] = a.transpose(0, 3, 4, 1, 2).reshape(n_st, 112, b_blocks, 128)
    xt[:, 112:] = 1.0
    return xt.reshape(n_st, 128, b_blocks * 128)


_PROGRAM_CACHE = {}


def _build_program(n_st=N_ST, b_blocks=B):
    """Build + compile the per-core Bass program (identical on all cores)."""
    import json
    key = (n_st, b_blocks, json.dumps(_build_program.consts, sort_keys=True, default=str))
    if key in _PROGRAM_CACHE:
        return _PROGRAM_CACHE[key]

    import concourse.bass as bass
    import concourse.tile as tile
    from concourse import bacc, mybir
    from contextlib import ExitStack

    f32 = mybir.dt.float32
    f16 = mybir.dt.float16
    Sin = mybir.ActivationFunctionType.Sin
    Square = mybir.ActivationFunctionType.Square
    Ident = mybir.ActivationFunctionType.Identity
    sub = mybir.AluOpType.subtract
    mult = mybir.AluOpType.mult
    addop = mybir.AluOpType.add

    nc = bacc.Bacc("TRN2", target_bir_lowering=False, debug=False,
                   num_devices=N_CORES)
    xh_d = nc.dram_tensor("xh", [n_st, 128, b_blocks * 128], f16,
                          kind="ExternalInput")
    wp_d = nc.dram_tensor("wp", [128, 84], f16, kind="ExternalInput")
    out_d = nc.dram_tensor("out", [n_st, 128, b_blocks, SLOTS], f32,
                           kind="ExternalOutput")

    L3, L2, L1 = _build_program.consts
    GROUP = 5

    def reg_const(v):
        if (f32, v) not in nc.const_aps.aps:
            t = nc.alloc_sbuf_tensor(
                f"constx-{len(nc.const_aps.aps)}", [128, 1], f32)
            nc.gpsimd.memset(t.ap(), v)
            nc.const_aps.aps[(f32, v)] = t.ap()

    reg_const(MAGIC)
    nc.all_engine_barrier()

    with tile.TileContext(nc) as tc:
        with ExitStack() as ctx:
            const_pool = ctx.enter_context(tc.tile_pool(name="const", bufs=1))
            xpool = ctx.enter_context(tc.tile_pool(name="x", bufs=2))
            ppool = ctx.enter_context(
                tc.tile_pool(name="ps", bufs=2, space=bass.MemorySpace.PSUM))
            gpool = ctx.enter_context(tc.tile_pool(name="g", bufs=2))

            wp = const_pool.tile([128, 84], f16)
            nc.sync.dma_start(wp[:], wp_d[:])

            st0 = 0
            while st0 < n_st:
                glen = min(GROUP, n_st - st0)
                q = glen * b_blocks
                ccols = {"q1g": 28, "q2g": 28, "k3g": 28, "f3g": 28,
                         "t3g": 28, "zg": 28, "o3acc": 28,
                         "s2": 14, "p2": 14, "k2": 14, "sc2": 14, "f2": 14,
                         "t2": 14, "u0g2": 14, "u1g2": 14, "o2": 14,
                         "s1": 7, "p1": 7, "k1": 7, "sc1": 7, "f1": 7,
                         "t1": 7, "u0g1": 7, "u1g1": 7, "yo": 7}

                def gt(nm, bufs=1):
                    c = ccols[nm]
                    t = gpool.tile([128, GROUP * b_blocks * c], f32,
                                   name=nm, tag=nm, bufs=bufs)
                    return t, t[:].rearrange("p (q c) -> p q c", c=c)

                q1g, q1gv = gt("q1g", bufs=2)
                q2g, q2gv = gt("q2g", bufs=2)
                k3g, k3gv = gt("k3g", bufs=2)
                f3g, f3gv = gt("f3g", bufs=2)
                o3acc, o3accv = gt("o3acc")

                for seg in range(glen):
                    st = st0 + seg
                    x2h = xpool.tile([128, b_blocks * 128], f16, name="x2h",
                                     tag="x2h")
                    nc.sync.dma_start(x2h[:], xh_d[st])

                    ps = ppool.tile([128, b_blocks * 128], f32)
                    for b in range(b_blocks):
                        nc.tensor.matmul(ps[:, 128 * b:128 * b + 84],
                                         x2h[:, 128 * b:128 * b + 128],
                                         wp[:], start=True, stop=True)
                    psv = ps[:].rearrange("p (b c) -> p b c", c=128)
                    segsl = slice(seg * b_blocks, (seg + 1) * b_blocks)

                    # evacuate PSUM: squares + k3 on ACT, f3 on DVE
                    nc.scalar.activation(q1gv[:, segsl, :], psv[:, :, 0:28],
                                         Square, bias=0.0, scale=1.0)
                    nc.scalar.activation(q2gv[:, segsl, :], psv[:, :, 28:56],
                                         Square, bias=0.0, scale=1.0)
                    nc.scalar.activation(k3gv[:, segsl, :], psv[:, :, 56:84],
                                         Ident, bias=MAGIC, scale=1.0)
                    nc.vector.scalar_tensor_tensor(
                        f3gv[:, segsl, :], k3gv[:, segsl, :], MAGIC,
                        psv[:, :, 56:84], sub, sub)

                # ---- level 3 (batched over the group) ----
                qf28 = q * 28
                t3g, t3gv = gt("t3g")
                nc.scalar.activation(t3g[:, 0:qf28], f3g[:, 0:qf28], Sin,
                                     bias=0.0, scale=NEG2PI)
                # L2 pairing: l2-run = [o3_0, o3_2], r2-run = [o3_1, o3_3]
                opos = {0: 0, 2: 7, 1: 14, 3: 21}
                qs = (q1gv, q2gv)
                zg, zgv = gt("zg")
                for n in range(4):
                    cn = L3[n]
                    sl = (slice(None), slice(0, q), slice(7 * n, 7 * n + 7))
                    qa = qs[cn["qa"]]
                    qb = qs[1 - cn["qa"]]
                    # z = R*t3 + qa ; o3 = z + K - qb
                    nc.vector.scalar_tensor_tensor(
                        zgv[sl], t3gv[sl], cn["R"], qa[sl], mult, addop)
                    nc.vector.scalar_tensor_tensor(
                        o3accv[:, 0:q, opos[n]:opos[n] + 7], zgv[sl],
                        cn["K"], qb[sl], addop, sub)

                # ---- level 2 (batched over the group) ----
                l2 = o3accv[:, 0:q, 0:14]
                r2 = o3accv[:, 0:q, 14:28]
                s2, s2f = gt("s2")
                s2v = s2f[:, 0:q, :]
                nc.gpsimd.tensor_add(s2v, l2, r2)
                p2, p2f = gt("p2")
                p2v = p2f[:, 0:q, :]
                nc.gpsimd.tensor_mul(p2v, l2, r2)
                qf = q * 14
                k2, _ = gt("k2")
                nc.vector.tensor_scalar(k2[:, 0:qf], s2[:, 0:qf], INV2PI,
                                        MAGIC, mult, addop)
                sc2, _ = gt("sc2")
                nc.vector.tensor_scalar_mul(sc2[:, 0:qf], s2[:, 0:qf], INV2PI)
                f2, _ = gt("f2")
                nc.vector.scalar_tensor_tensor(f2[:, 0:qf], k2[:, 0:qf], MAGIC,
                                               sc2[:, 0:qf], sub, sub)
                t2, t2f = gt("t2")
                nc.scalar.activation(t2[:, 0:qf], f2[:, 0:qf], Sin, bias=0.0,
                                     scale=NEG2PI)
                u0g2, u0g2f = gt("u0g2")
                u1g2, u1g2f = gt("u1g2")
                o2, o2f = gt("o2")
                o2v = o2f[:, 0:q, :]
                for m in range(2):
                    cm = L2[m]
                    sl = (slice(None), slice(0, q), slice(7 * m, 7 * m + 7))
                    nc.vector.tensor_scalar(u0g2f[sl], p2f[sl], cm["P"],
                                            cm["beta"], mult, addop)
                    nc.vector.scalar_tensor_tensor(
                        u1g2f[sl], t2f[sl], cm["R"], u0g2f[sl], mult, addop)
                    nc.vector.scalar_tensor_tensor(
                        o2f[sl], s2f[sl], cm["A"], u1g2f[sl], mult, addop)

                # ---- level 1 ----
                l1 = o2v[:, :, 0:7]
                r1 = o2v[:, :, 7:14]
                qf = q * 7
                s1, s1f = gt("s1")
                s1v = s1f[:, 0:q, :]
                nc.gpsimd.tensor_add(s1v, l1, r1)
                p1, p1f = gt("p1")
                p1v = p1f[:, 0:q, :]
                nc.gpsimd.tensor_mul(p1v, l1, r1)
                k1, _ = gt("k1")
                nc.vector.tensor_scalar(k1[:, 0:qf], s1[:, 0:qf], INV2PI,
                                        MAGIC, mult, addop)
                sc1, _ = gt("sc1")
                nc.vector.tensor_scalar_mul(sc1[:, 0:qf], s1[:, 0:qf], INV2PI)
                f1, _ = gt("f1")
                nc.vector.scalar_tensor_tensor(f1[:, 0:qf], k1[:, 0:qf], MAGIC,
                                               sc1[:, 0:qf], sub, sub)
                t1, t1f = gt("t1")
                t1v = t1f[:, 0:q, :]
                nc.scalar.activation(t1[:, 0:qf], f1[:, 0:qf], Sin, bias=0.0,
                                     scale=NEG2PI)
                c1 = L1[0]
                u0g1, u0g1f = gt("u0g1")
                u0_1v = u0g1f[:, 0:q, :]
                nc.vector.tensor_scalar(u0_1v, p1v, c1["P"], c1["beta"],
                                        mult, addop)
                u1g1, u1g1f = gt("u1g1")
                u1_1v = u1g1f[:, 0:q, :]
                nc.vector.scalar_tensor_tensor(
                    u1_1v, t1v, c1["R"], u0_1v, mult, addop)
                yo, yof = gt("yo")
                yov = yof[:, 0:q, :]
                nc.vector.scalar_tensor_tensor(
                    yov, s1v, c1["A"], u1_1v, mult, addop)

                dst = out_d[st0:st0 + glen].transpose([1, 0, 2, 3])
                yo4 = yo[:, 0:qf].rearrange("p (g b a) -> p g b a",
                                            g=glen, a=SLOTS)
                nc.sync.dma_start(dst, yo4)
                st0 += glen

    nc.compile()
    _PROGRAM_CACHE[key] = nc
    return nc


def kernel(x, leaf_w, leaf_b, w1, b1, om1, w2, b2, om2, w3, b3, om3):
    from concourse.bass_interp import get_hw_module
    from concourse.bass_utils import run_bass_kernel_spmd

    L3, L2, L1, wp = _fold(leaf_w, leaf_b, w1, b1, om1, w2, b2, om2, w3, b3, om3)
    _build_program.consts = (L3, L2, L1)
    nc = _build_program()

    in_maps = []
    x = np.ascontiguousarray(x, dtype=F32)
    for c in range(N_CORES):
        xh = _pack_x(x[c * N_CORE:(c + 1) * N_CORE])
        in_maps.append({"xh": xh, "wp": wp})

    kw = {}
    if os.environ.get("KERNEL_TRACE_DIR"):
        kw["tmpdir"] = os.environ["KERNEL_TRACE_DIR"]
    old = nc.m
    nc.m = get_hw_module(nc.m)
    try:
        res = run_bass_kernel_spmd(nc, in_maps, core_ids=list(range(N_CORES)), **kw)
    finally:
        nc.m = old
    kernel._last = res

    out = np.empty(N_FULL, F32)
    for c in range(N_CORES):
        oc = res.results[c]["out"]          # [N_ST, 128, B, 7]
        oc = oc.transpose(0, 2, 1, 3).reshape(-1)[:N_CORE]
        out[c * N_CORE:(c + 1) * N_CORE] = oc
    return out
